# revision 42
# baseline (speedup 1.0000x reference)
"""GQA causal attention (B=4, S=2048, D=2048, H=16, KVH=8, HD=128) on 8 trn2 cores.

Sharding: batch x head-group. Core c = (b, g) with b = c // 2, g = c % 2.
Each core handles one batch and half the heads (8 q-heads, 4 kv-heads),
computing a partial output out_partial = attn_out_g @ wo_g for its batch.
Host sums the two partials per batch (row-sharded wo => partial sums).

Device kernel (per core, identical SPMD program): single pass over the
sequence in 512-wide chunks. Causality means chunk qc's attention only
needs K/V for chunks <= qc, which are all projected by then, so the
K/V/Q projections, rope, attention, and output projection for a chunk
are emitted together and pipelined by the Tile scheduler.

  - All matmuls in float32r (fp32 data, PE truncates to ~fp22, full rate).
  - Scores computed transposed (ST[k, q]) so softmax probs come out of the
    exp in the [k, q] layout PV needs -- no P transposes.
  - Softmax without max subtraction (|scale*scores| < ~10, exp is safe);
    column sums via ones-matmul (broadcast across partitions), reciprocal,
    normalization applied on the small PV output, not on P.
  - Causal mask on diagonal blocks via gpsimd.affine_select (zero fill
    after exp).
"""

import numpy as np

D = 2048
S = 2048
HQ = 8        # q heads per core
HKV = 4       # kv heads per core
HD = 128
KK = D // 128         # 16 contraction subtiles
QC = S // 512         # 4 sequence chunks of 512
NST = S // 128        # 16 sequence tiles of 128
SCALE = 1.0 / float(np.sqrt(HD))

_CACHE = {}


def _swap_mask():
    m = []
    for i in range(16):
        m += [2 * i + 1, 2 * i]
    return m


def build_nc():
    """Builds and compiles the per-core Bass program. Returns the Bacc."""
    from contextlib import ExitStack

    import concourse.mybir as mybir
    import concourse.tile as tile
    from concourse import bacc

    f32 = mybir.dt.float32
    f32r = mybir.dt.float32r
    AF = mybir.ActivationFunctionType
    OP = mybir.AluOpType

    nc = bacc.Bacc(None, target_bir_lowering=False)

    xT = nc.dram_tensor("xT", [D, S], f32, kind="ExternalInput")
    wq = nc.dram_tensor("wq", [D, HQ * HD], f32, kind="ExternalInput")
    wk = nc.dram_tensor("wk", [D, HKV * HD], f32, kind="ExternalInput")
    wv = nc.dram_tensor("wv", [D, HKV * HD], f32, kind="ExternalInput")
    wo = nc.dram_tensor("wo", [HQ * HD, D], f32, kind="ExternalInput")
    cosb = nc.dram_tensor("cosb", [HD, S], f32, kind="ExternalInput")
    sinb = nc.dram_tensor("sinb", [HD, S], f32, kind="ExternalInput")
    out = nc.dram_tensor("out", [S, D], f32, kind="ExternalOutput")

    SWAP = _swap_mask()

    with tile.TileContext(nc) as tc, ExitStack() as ctx:
        constp = ctx.enter_context(tc.tile_pool(name="constp", bufs=1))
        tabs = ctx.enter_context(tc.tile_pool(name="tabs", bufs=1))
        kvp = ctx.enter_context(tc.tile_pool(name="kvp", bufs=1))
        vstp = ctx.enter_context(tc.tile_pool(name="vstp", bufs=1))
        xsp = ctx.enter_context(tc.tile_pool(name="xsp", bufs=16))
        tmpp = ctx.enter_context(tc.tile_pool(name="tmpp", bufs=5))
        qtp = ctx.enter_context(tc.tile_pool(name="qtp", bufs=8))
        ptp = ctx.enter_context(tc.tile_pool(name="ptp", bufs=3))
        onp = ctx.enter_context(tc.tile_pool(name="onp", bufs=8))
        rbp = ctx.enter_context(tc.tile_pool(name="rbp", bufs=2))
        oevp = ctx.enter_context(tc.tile_pool(name="oevp", bufs=2))
        wkp = ctx.enter_context(tc.tile_pool(name="wkp", bufs=4))
        wvp = ctx.enter_context(tc.tile_pool(name="wvp", bufs=4))
        wqp = ctx.enter_context(tc.tile_pool(name="wqp", bufs=4))
        wop = ctx.enter_context(tc.tile_pool(name="wop", bufs=8))
        psA = ctx.enter_context(tc.tile_pool(name="psA", bufs=4, space="PSUM"))
        psS = ctx.enter_context(tc.tile_pool(name="psS", bufs=2, space="PSUM"))
        psO = ctx.enter_context(tc.tile_pool(name="psO", bufs=1, space="PSUM"))
        psL = ctx.enter_context(tc.tile_pool(name="psL", bufs=1, space="PSUM"))

        ones_f = constp.tile([128, 128], f32, name="ones_f")
        nc.vector.memset(ones_f[:], 1.0)
        ones = constp.tile([128, 128], f32r, name="ones")
        nc.scalar.copy(ones[:], ones_f[:])

        # Persistent K^T (rotated) per kv head and V tiles.
        KT = [kvp.tile([HD, S], f32r, name=f"kt{h}") for h in range(HKV)]
        V = [vstp.tile([128, HKV * HD], f32r, name=f"v{st}") for st in range(NST)]

        def rope(psrc, ct, st, dst):
            """dst = psrc*cos + pairswap(psrc)*sin.

            ACT evicts the PSUM bank in one op (fast release); DVE does the
            rotation from SBUF, clobbering the staging tile in place.
            """
            t1 = tmpp.tile([128, 512], f32, name="rope_t1")
            nc.scalar.copy(t1[:], psrc[:])
            sw = tmpp.tile([128, 512], f32, name="rope_sw")
            nc.vector.stream_shuffle(sw[:], t1[:], SWAP)
            nc.vector.tensor_mul(sw[:], sw[:], st[:])
            nc.vector.tensor_mul(t1[:], t1[:], ct[:])
            nc.vector.tensor_add(dst, t1[:], sw[:])

        def load_chunk(sc):
            ssl = slice(sc * 512, (sc + 1) * 512)
            xa = []
            wk_tiles = {}
            for kk in range(KK):
                t = xsp.tile([128, 512], f32r, name="xs")
                nc.sync.dma_start(
                    t[:], xT[kk * 128:(kk + 1) * 128, ssl].bitcast(f32r)
                )
                xa.append(t)
                if kk < 2:
                    wkt = wkp.tile([128, HKV * HD], f32r, name="wkt")
                    nc.sync.dma_start(
                        wkt[:], wk[kk * 128:(kk + 1) * 128, :].bitcast(f32r)
                    )
                    wk_tiles[kk] = wkt
            ct = tabs.tile([HD, 512], f32, name="cos_sl")
            nc.sync.dma_start(ct[:], cosb[:, ssl])
            stt = tabs.tile([HD, 512], f32, name="sin_sl")
            nc.sync.dma_start(stt[:], sinb[:, ssl])
            return xa, wk_tiles, ct, stt

        loads = load_chunk(0)
        for sc in range(QC):
            ssl = slice(sc * 512, (sc + 1) * 512)
            xa, wk_tiles, ct, stt = loads

            # K^T projection: 4 kv heads accumulated in 4 PSUM banks.
            pss = [psA.tile([128, 512], f32, name="psA") for _ in range(HKV)]
            for kk in range(KK):
                if kk in wk_tiles:
                    wkt = wk_tiles[kk]
                else:
                    wkt = wkp.tile([128, HKV * HD], f32r, name="wkt")
                    nc.sync.dma_start(
                        wkt[:], wk[kk * 128:(kk + 1) * 128, :].bitcast(f32r)
                    )
                for h in range(HKV):
                    nc.tensor.matmul(
                        pss[h][:],
                        wkt[:, h * HD:(h + 1) * HD],
                        xa[kk][:],
                        start=(kk == 0),
                        stop=(kk == KK - 1),
                    )
            for h in range(HKV):
                rope(pss[h], ct, stt, KT[h][:, ssl])

            # V projection: 4 sequence tiles accumulated in 4 PSUM banks.
            psv = [psA.tile([128, 512], f32, name="psA") for _ in range(4)]
            for kk in range(KK):
                wvt = wvp.tile([128, HKV * HD], f32r, name="wvt")
                nc.sync.dma_start(
                    wvt[:], wv[kk * 128:(kk + 1) * 128, :].bitcast(f32r)
                )
                for st in range(4):
                    nc.tensor.matmul(
                        psv[st][:],
                        xa[kk][:, st * 128:(st + 1) * 128],
                        wvt[:],
                        start=(kk == 0),
                        stop=(kk == KK - 1),
                    )
            for st in range(4):
                nc.scalar.copy(V[sc * 4 + st][:], psv[st][:])

            # Q^T projection + rope, heads in groups of 4 (4 PSUM banks).
            QTr = []
            for hg in range(2):
                psq = [psA.tile([128, 512], f32, name="psA") for _ in range(4)]
                for kk in range(KK):
                    wqt = wqp.tile([128, 512], f32r, name="wqt")
                    nc.sync.dma_start(
                        wqt[:],
                        wq[kk * 128:(kk + 1) * 128, hg * 512:(hg + 1) * 512].bitcast(f32r),
                    )
                    for j in range(4):
                        nc.tensor.matmul(
                            psq[j][:],
                            wqt[:, j * HD:(j + 1) * HD],
                            xa[kk][:],
                            start=(kk == 0),
                            stop=(kk == KK - 1),
                        )
                for j in range(4):
                    qt = qtp.tile([128, 512], f32r, name="qt")
                    rope(psq[j], ct, stt, qt[:])
                    QTr.append(qt)

            if sc + 1 < QC:
                loads = load_chunk(sc + 1)

            # Attention for each head on this chunk (keys 0 .. chunk end).
            nk = 4 * (sc + 1)
            ON = []
            for h in range(HQ):
                kvh = h // 2
                po = psO.tile([128, 512], f32, name="psO")
                pl = psL.tile([128, 512], f32, name="psL")
                for kt in range(nk):
                    # Diagonal blocks only need query columns >= 128*j.
                    j = kt - 4 * sc
                    off = 128 * j if j > 0 else 0
                    nj = 512 - off
                    ss = psS.tile([128, 512], f32, name="psS")
                    nc.tensor.matmul(
                        ss[:, :nj],
                        KT[kvh][:, kt * 128:(kt + 1) * 128],
                        QTr[h][:, off:],
                        start=True,
                        stop=True,
                    )
                    pt = ptp.tile([128, 512], f32r, name="pt")
                    nc.scalar.activation(pt[:, :nj], ss[:, :nj], AF.Exp, scale=SCALE)
                    if j >= 0:
                        nc.gpsimd.affine_select(
                            out=pt[:, :nj],
                            in_=pt[:, :nj],
                            compare_op=OP.is_ge,
                            fill=0.0,
                            base=0,
                            pattern=[[1, nj]],
                            channel_multiplier=-1,
                        )
                    nc.tensor.matmul(
                        po[:, off:],
                        V[kt][:, kvh * HD:(kvh + 1) * HD],
                        pt[:, :nj],
                        start=(kt == 0),
                        stop=(kt == nk - 1),
                    )
                    nc.tensor.matmul(
                        pl[:, off:],
                        ones[:],
                        pt[:, :nj],
                        start=(kt == 0),
                        stop=(kt == nk - 1),
                    )
                rb = rbp.tile([128, 512], f32, name="rb")
                nc.vector.reciprocal(rb[:], pl[:])
                on = onp.tile([128, 512], f32r, name="on")
                nc.vector.tensor_mul(on[:], po[:], rb[:])
                ON.append(on)

            # Output projection for this chunk.
            for dc in range(4):
                dsl = slice(dc * 512, (dc + 1) * 512)
                wo_ts = []
                for h in range(HQ):
                    wt = wop.tile([128, 512], f32r, name="wot")
                    nc.sync.dma_start(
                        wt[:], wo[h * HD:(h + 1) * HD, dsl].bitcast(f32r)
                    )
                    wo_ts.append(wt)
                for qs in range(4):
                    ps = psA.tile([128, 512], f32, name="psA")
                    for h in range(HQ):
                        nc.tensor.matmul(
                            ps[:],
                            ON[h][:, qs * 128:(qs + 1) * 128],
                            wo_ts[h][:],
                            start=(h == 0),
                            stop=(h == HQ - 1),
                        )
                    oev = oevp.tile([128, 512], f32, name="oev")
                    if qs % 2 == 0:
                        nc.scalar.copy(oev[:], ps[:])
                    else:
                        nc.vector.tensor_copy(oev[:], ps[:])
                    r0 = sc * 512 + qs * 128
                    nc.sync.dma_start(out[r0:r0 + 128, dsl], oev[:])

    nc.compile()
    return nc


def _get_nc():
    if "nc" not in _CACHE:
        _CACHE["nc"] = build_nc()
    return _CACHE["nc"]


def _host_prep(x, wq, wk, wv, wo, pos_cos, pos_sin):
    x = np.asarray(x, dtype=np.float32)
    wq = np.asarray(wq, dtype=np.float32)
    wk = np.asarray(wk, dtype=np.float32)
    wv = np.asarray(wv, dtype=np.float32)
    wo = np.asarray(wo, dtype=np.float32)
    pos_cos = np.asarray(pos_cos, dtype=np.float32)
    pos_sin = np.asarray(pos_sin, dtype=np.float32)

    cosb = np.repeat(pos_cos.T, 2, axis=0).copy()          # [128, S]
    sinb = np.repeat(pos_sin.T, 2, axis=0).copy()          # [128, S]
    sinb[0::2, :] *= -1.0

    in_maps = []
    for c in range(8):
        b, g = c // 2, c % 2
        in_maps.append({
            "xT": np.ascontiguousarray(x[b].T),
            "wq": np.ascontiguousarray(wq[:, g * 1024:(g + 1) * 1024]),
            "wk": np.ascontiguousarray(wk[:, g * 512:(g + 1) * 512]),
            "wv": np.ascontiguousarray(wv[:, g * 512:(g + 1) * 512]),
            "wo": np.ascontiguousarray(wo[g * 1024:(g + 1) * 1024, :]),
            "cosb": cosb,
            "sinb": sinb,
        })
    return in_maps


def kernel(x, wq, wk, wv, wo, pos_cos, pos_sin):
    from concourse.bass_utils import run_bass_kernel_spmd

    nc = _get_nc()
    in_maps = _host_prep(x, wq, wk, wv, wo, pos_cos, pos_sin)
    res = run_bass_kernel_spmd(nc, in_maps, core_ids=list(range(8)))
    outs = [r["out"] for r in res.results]
    full = np.empty((4, S, D), dtype=np.float32)
    for b in range(4):
        full[b] = outs[2 * b] + outs[2 * b + 1]
    return full
